# revision 5
# baseline (speedup 1.0000x reference)
"""ARD RBF Gram matrix kernel for Trainium2 (8 NeuronCores, SPMD).

K[i, j] = exp(-0.5 * sum_d (x[i,d] - y[j,d])^2 / exp(logh[d]))

Sharding: 2x4 core grid. Core c = (r, q) with r = c // 4, q = c % 4 owns the
output block rows [r*4096, (r+1)*4096) x cols [q*2048, (q+1)*2048). This
minimizes per-core input DMA (8MB of x + 4MB of y vs 2MB + 16MB for pure row
sharding).

Device-side algorithm per core:
  ih      = exp(-0.5 * logh)                       (ACT)
  xs8     = fp8e4(x^T * ih)     [d, c, i] layout   (DVE per-partition scale)
  ys8     = fp8e4(y^T * -2ih)   [d, c, j] layout   (DVE per-partition scale)
  mhx2[i] = -0.5*sum_d ih^2 x^2  via f32r matmul reduce + transpose DMA
  y2[j]   =      sum_d ih^2 y^2  via f32r matmul reduce, bf16 hi/lo split
  psum    = sum_d xs8^T.T @ ys8  (fp8 DoubleRow matmuls, 256-deep passes)
            + y2 hi/lo           (bf16 aug matmul rows)
  out     = exp(-0.5*psum + mhx2[i])  (ACT, PSUM -> SBUF fp16)
  DMA store fp16 to DRAM; host widens fp16 -> fp32 (lossless).

The host side only reshapes/transposes/shards numpy arrays and losslessly
widens the fp16 result; every value-changing floating point operation happens
on device.
"""

import json

import numpy as np

import concourse.bass as bass
import concourse.mybir as mybir
import concourse.tile as tile
from concourse.bass_utils import run_bass_kernel_spmd

N_CORES = 8
N, M, D = 8192, 8192, 512
RG, CG = 2, 4  # core grid: RG row groups x CG col groups
NI = N // RG  # x rows per core (4096)
MJ = M // CG  # y cols per core (2048)
P = 128  # partitions
NCHUNK = D // P  # contraction chunks (4)
NPAIR = NCHUNK // 2  # fp8 DoubleRow chunk pairs (2)
ITILES = NI // P  # i tiles per core (32)
JT = MJ // 512  # 512-wide j strips per core (4)

F32 = mybir.dt.float32
F32R = mybir.dt.float32r
BF16 = mybir.dt.bfloat16
F16 = mybir.dt.float16
FP8 = mybir.dt.float8e4
AF = mybir.ActivationFunctionType
DR = mybir.MatmulPerfMode.DoubleRow

# ---------------------------------------------------------------------------
# Workaround for this walrus build: only ONE sync-wait condition is allowed
# per instruction ("Too many sync wait commands"). Split excess on_wait
# entries onto preceding NoOps on the same engine (program order preserves
# semantics exactly).
# ---------------------------------------------------------------------------
_WAIT_LIMIT = 1


def _split_excess_waits(bir: dict, limit: int = _WAIT_LIMIT) -> dict:
    # Excess waits are moved onto preceding EventSemaphore instructions,
    # which this walrus accepts with up to TWO wait conditions (ordinary
    # instructions allow only one). Program order preserves semantics.
    counter = 0
    for fn in bir.get("functions", []):
        for bb in fn.get("blocks", []):
            new_insts = []
            for inst in bb.get("instructions", []):
                si = inst.get("sync_info")
                waits = si.get("on_wait") if si else None
                eng = inst.get("engine", "Unassigned")
                if waits and len(waits) > limit and eng != "Unassigned":
                    keep = len(waits) % 2  # odd count: last wait stays put
                    head = waits[: len(waits) - keep]
                    for i in range(0, len(head), 2):
                        counter += 1
                        new_insts.append(
                            {
                                "debug": inst.get("debug", 0),
                                "engine": eng,
                                "ins": [],
                                "outs": [],
                                "name": f"WS-{counter}-{inst['name']}",
                                "opcode": "EventSemaphore",
                                "sync_info": {
                                    "on_update": [],
                                    "on_wait": head[i : i + 2],
                                },
                            }
                        )
                    si["on_wait"] = waits[len(waits) - keep :]
                new_insts.append(inst)
            bb["instructions"] = new_insts
    return bir


def _patch_nc(nc):
    orig = nc.to_json_bytes

    def patched() -> bytes:
        return json.dumps(_split_excess_waits(json.loads(orig()))).encode()

    nc.to_json_bytes = patched
    return nc


# ---------------------------------------------------------------------------
# Device program (identical on all 8 cores; only DRAM contents differ)
# ---------------------------------------------------------------------------


def _build_nc():
    nc = bass.Bass()

    xt = nc.dram_tensor("xt", [D, NI], F32, kind="ExternalInput")
    yt = nc.dram_tensor("yt", [D, MJ], F32, kind="ExternalInput")
    lh = nc.dram_tensor("lh", [NCHUNK, P], F32, kind="ExternalInput")
    out = nc.dram_tensor("out", [NI, MJ], F16, kind="ExternalOutput")

    xt_r = xt.rearrange("(c d) i -> d c i", d=P)
    yt_r = yt.rearrange("(c d) j -> d c j", d=P)

    with tile.TileContext(nc) as tc:
        with (
            tc.tile_pool(name="singles", bufs=1) as singles,
            tc.tile_pool(name="xstage", bufs=2) as xstage,
            tc.tile_pool(name="ystage", bufs=2) as ystage,
            tc.tile_pool(name="sqp", bufs=2) as sqp,
            tc.tile_pool(name="stp", bufs=2) as stp,
            tc.tile_pool(name="accp", bufs=2, space="PSUM") as accp,
            tc.tile_pool(name="outp", bufs=3) as outp,
            tc.tile_pool(name="mainps", bufs=3, space="PSUM") as mainps,
        ):
            # persistent SBUF tensors
            xs8 = singles.tile([P, NCHUNK, NI], FP8)  # ih * x^T, fp8
            ys8 = singles.tile([P, NCHUNK, MJ], FP8)  # -2 ih * y^T, fp8
            aug_r = singles.tile([2, MJ], BF16)  # rows: y2_hi, y2_lo
            ones2 = singles.tile([2, P], BF16)  # aug lhsT (all ones)
            mhx2 = singles.tile([P, ITILES], F32)  # -0.5 * x2, ACT bias
            sx = singles.tile([1, NI], F32)  # -0.5 * x2 row accum
            sy = singles.tile([1, MJ], F32)  # y2 row accum
            lhs = singles.tile([P, NCHUNK], F32)
            ih = singles.tile([P, NCHUNK], F32)
            ihm2 = singles.tile([P, NCHUNK], F32)
            ihsq = singles.tile([P, NCHUNK], F32)
            mihsq = singles.tile([P, NCHUNK], F32)

            nc.sync.dma_start(out=lhs, in_=lh.rearrange("c d -> d c"))
            nc.scalar.activation(ih, lhs, AF.Exp, scale=-0.5)
            nc.vector.tensor_scalar_mul(ihm2, ih, -2.0)
            # f32r-tagged writes: the BIR verifier requires every operand of
            # an fp32r matmul to be produced as fp32r.
            nc.vector.tensor_mul(ihsq.bitcast(F32R), ih, ih)
            nc.vector.tensor_scalar_mul(mihsq.bitcast(F32R), ihsq, -0.5)
            nc.gpsimd.memset(ones2, 1.0)

            def reduce_rows(src, w, lhsT, dst, c, pfx):
                # dst[0, :w] (SBUF f32) += sum_d lhsT[d] * src[d, :w]^2 for
                # one contraction chunk c (start/stop per matmul, DVE accum).
                sq = sqp.tile([P, w], F32, tag=f"{pfx}sq", name=f"{pfx}sq{c}")
                nc.vector.tensor_mul(sq.bitcast(F32R), src, src)
                for js in range(w // 512):
                    sl = slice(js * 512, (js + 1) * 512)
                    acc = accp.tile(
                        [1, 512], F32, tag="acc", name=f"{pfx}a{c}_{js}"
                    )
                    nc.tensor.matmul(
                        acc,
                        lhsT.bitcast(F32R),
                        sq[:, sl].bitcast(F32R),
                        start=True,
                        stop=True,
                    )
                    if c == 0:
                        nc.vector.tensor_copy(dst[0:1, sl], acc)
                    else:
                        nc.vector.tensor_add(dst[0:1, sl], dst[0:1, sl], acc)

            # ---- y prep: load f32 chunk, y2 reduce, scale to fp8 ----
            for c in range(NCHUNK):
                yf = ystage.tile([P, MJ], F32, tag="y", name=f"y{c}")
                nc.sync.dma_start(out=yf, in_=yt_r[:, c, :])
                reduce_rows(yf, MJ, ihsq[:, c : c + 1], sy, c, "y")
                nc.vector.tensor_scalar_mul(
                    ys8[:, c, :], yf, ihm2[:, c : c + 1]
                )

            # y2 -> bf16 hi/lo aug rows (row 0 via DVE at partition 0; row 1
            # via DMA because engine APs must start at a 32-aligned partition)
            nc.vector.tensor_copy(aug_r[0:1, :], sy)
            stl = stp.tile([1, MJ], BF16, tag="stl", name="stl")
            nc.vector.tensor_sub(stl, sy, aug_r[0:1, :])
            nc.sync.dma_start(out=aug_r[1:2, :], in_=stl)

            # ---- x prep: load f32 chunk, -0.5*x2 reduce, scale to fp8 ----
            for c in range(NCHUNK):
                xf = xstage.tile([P, NI], F32, tag="x", name=f"x{c}")
                nc.sync.dma_start(out=xf, in_=xt_r[:, c, :])
                reduce_rows(xf, NI, mihsq[:, c : c + 1], sx, c, "x")
                nc.vector.tensor_scalar_mul(xs8[:, c, :], xf, ih[:, c : c + 1])

            # transpose -0.5*x2 row [1, NI] -> [P, ITILES] for the ACT bias:
            # one column DMA per itile (a single balanced DMA would need >3
            # AP dims, which the DMA engine can't express).
            for it in range(ITILES):
                nc.sync.dma_start(
                    out=mhx2[:, it : it + 1],
                    in_=sx[0:1, it * P : (it + 1) * P],
                )

            # ---- main loop: fp8 DoubleRow matmuls + bf16 aug + ACT exp ----
            for it in range(ITILES):
                isl = slice(it * P, (it + 1) * P)
                ot = outp.tile([P, MJ], F16, tag="ot", name=f"ot{it}")
                for jg in range(JT // 2):
                    ps = mainps.tile(
                        [P, 1024], F32, tag="ps", name=f"ps{it}_{jg}"
                    )
                    for t in range(NPAIR):
                        csl = slice(2 * t, 2 * t + 2)
                        for js in range(2):
                            j0 = jg * 1024 + js * 512
                            nc.tensor.matmul(
                                ps[:, js * 512 : (js + 1) * 512],
                                xs8[:, csl, isl],
                                ys8[:, csl, j0 : j0 + 512],
                                start=(t == 0),
                                stop=False,
                                perf_mode=DR,
                            )
                    for js in range(2):
                        j0 = jg * 1024 + js * 512
                        nc.tensor.matmul(
                            ps[:, js * 512 : (js + 1) * 512],
                            ones2,
                            aug_r[:, j0 : j0 + 512],
                            start=False,
                            stop=True,
                        )
                    nc.scalar.activation(
                        ot[:, jg * 1024 : (jg + 1) * 1024],
                        ps,
                        AF.Exp,
                        bias=mhx2[:, it : it + 1],
                        scale=-0.5,
                    )
                nc.sync.dma_start(out=out[isl, :], in_=ot)

    return _patch_nc(nc)


_NC_CACHE = None

# test.py hooks: set _TRACE to capture a profile; results object stored here.
_TRACE = False
_TRACE_KWARGS = {}
LAST_RESULTS = None


def kernel(x, y, logh):
    global _NC_CACHE, LAST_RESULTS
    x = np.ascontiguousarray(np.asarray(x, dtype=np.float32))
    y = np.ascontiguousarray(np.asarray(y, dtype=np.float32))
    logh = np.ascontiguousarray(np.asarray(logh, dtype=np.float32))
    assert x.shape == (N, D) and y.shape == (M, D) and logh.shape == (D,)

    if _NC_CACHE is None:
        _NC_CACHE = _build_nc()
    nc = _NC_CACHE

    lhm = np.ascontiguousarray(logh.reshape(NCHUNK, P))
    xts = [
        np.ascontiguousarray(x[r * NI : (r + 1) * NI, :].T) for r in range(RG)
    ]
    yts = [
        np.ascontiguousarray(y[q * MJ : (q + 1) * MJ, :].T) for q in range(CG)
    ]
    in_maps = []
    for c in range(N_CORES):
        r, q = divmod(c, CG)
        in_maps.append({"xt": xts[r], "yt": yts[q], "lh": lhm})

    res = run_bass_kernel_spmd(
        nc,
        in_maps,
        core_ids=list(range(N_CORES)),
        trace=_TRACE,
        **_TRACE_KWARGS,
    )
    LAST_RESULTS = res
    full = np.empty((N, M), dtype=np.float32)
    for c in range(N_CORES):
        r, q = divmod(c, CG)
        full[r * NI : (r + 1) * NI, q * MJ : (q + 1) * MJ] = res.results[c][
            "out"
        ].astype(np.float32)
    return full


# revision 11
# speedup vs baseline: 1.2196x; 1.2196x over previous
"""ARD RBF Gram matrix kernel for Trainium2 (8 NeuronCores, SPMD).

K[i, j] = exp(-0.5 * sum_d (x[i,d] - y[j,d])^2 / exp(logh[d]))

Sharding: 2x4 core grid. Core c = (r, q) with r = c // 4, q = c % 4 owns the
output block rows [r*4096, (r+1)*4096) x cols [q*2048, (q+1)*2048). This
minimizes per-core input DMA (8MB of x + 4MB of y vs 2MB + 16MB for pure row
sharding).

Device-side algorithm per core, using the factorization
  K = exp(-0.5*cross - 0.5*x2[i]) * exp(-0.5*y2[j]),  cross = -2 sum ih^2 x y:

  ih      = exp(-0.5 * logh)                       (ACT)
  xs8     = fp8e4(x^T * ih)     [d, c, i] layout   (DVE per-partition scale)
  ys8     = fp8e4(y^T * -2ih)   [d, c, j] layout   (DVE per-partition scale)
  mhx2[i] = -0.5*sum_d ih^2 x^2  (f32r reduce matmuls; ACT bias, transposed
                                  into [128, ITILES] via small DMAs)
  ey2[j]  = exp(-0.5*sum_d ih^2 y^2)  (f32r reduce + ACT exp), replicated to
                                  all 128 partitions via a ones-matmul
  psum    = cross  (fp8 DoubleRow matmuls, 256-deep contraction per pass)
  tmp     = exp(-0.5*psum + mhx2[i])   (ACT, PSUM -> SBUF fp16)
  out     = tmp * ey2rep               (Pool engine fp16 multiply)
  DMA store fp16 to DRAM; host widens fp16 -> fp32 (lossless).

The aug matmuls of the v1 design are gone: they cost as much PE streaming
time as the real DR matmuls and their weight thrash kept the PE from ramping
to 2.4 GHz. The host side only reshapes/transposes/shards numpy arrays and
losslessly widens the fp16 result; every value-changing floating point
operation happens on device.
"""

import json

import numpy as np

import concourse.bass as bass
import concourse.mybir as mybir
import concourse.tile as tile
from concourse.bass_utils import run_bass_kernel_spmd

N_CORES = 8
N, M, D = 8192, 8192, 512
RG, CG = 2, 4  # core grid: RG row groups x CG col groups
NI = N // RG  # x rows per core (4096)
MJ = M // CG  # y cols per core (2048)
P = 128  # partitions
NCHUNK = D // P  # contraction chunks (4)
NPAIR = NCHUNK // 2  # fp8 DoubleRow chunk pairs (2)
ITILES = NI // P  # i tiles per core (32)
XSLABS = NI // 512  # x prep slabs (8)
YSLABS = MJ // 512  # y prep slabs (4)

F32 = mybir.dt.float32
F32R = mybir.dt.float32r
BF16 = mybir.dt.bfloat16
F16 = mybir.dt.float16
FP8 = mybir.dt.float8e4
AF = mybir.ActivationFunctionType
DR = mybir.MatmulPerfMode.DoubleRow

# ---------------------------------------------------------------------------
# Workaround for this walrus build: only ONE sync-wait condition is allowed
# per instruction ("Too many sync wait commands"). Split excess on_wait
# entries onto preceding NoOps on the same engine (program order preserves
# semantics exactly).
# ---------------------------------------------------------------------------
_WAIT_LIMIT = 1


def _split_excess_waits(bir: dict, limit: int = _WAIT_LIMIT) -> dict:
    # Excess waits are moved onto preceding EventSemaphore instructions,
    # which this walrus accepts with up to TWO wait conditions (ordinary
    # instructions allow only one). Program order preserves semantics.
    counter = 0
    for fn in bir.get("functions", []):
        for bb in fn.get("blocks", []):
            new_insts = []
            for inst in bb.get("instructions", []):
                si = inst.get("sync_info")
                waits = si.get("on_wait") if si else None
                eng = inst.get("engine", "Unassigned")
                if waits and len(waits) > limit and eng != "Unassigned":
                    keep = len(waits) % 2  # odd count: last wait stays put
                    head = waits[: len(waits) - keep]
                    for i in range(0, len(head), 2):
                        counter += 1
                        new_insts.append(
                            {
                                "debug": inst.get("debug", 0),
                                "engine": eng,
                                "ins": [],
                                "outs": [],
                                "name": f"WS-{counter}-{inst['name']}",
                                "opcode": "EventSemaphore",
                                "sync_info": {
                                    "on_update": [],
                                    "on_wait": head[i : i + 2],
                                },
                            }
                        )
                    si["on_wait"] = waits[len(waits) - keep :]
                new_insts.append(inst)
            bb["instructions"] = new_insts
    return bir


def _patch_nc(nc):
    orig = nc.to_json_bytes

    def patched() -> bytes:
        return json.dumps(_split_excess_waits(json.loads(orig()))).encode()

    nc.to_json_bytes = patched
    return nc


# ---------------------------------------------------------------------------
# Device program (identical on all 8 cores; only DRAM contents differ)
# ---------------------------------------------------------------------------


def _build_nc():
    nc = bass.Bass()

    xt = nc.dram_tensor("xt", [D, NI], F32, kind="ExternalInput")
    yt = nc.dram_tensor("yt", [D, MJ], F32, kind="ExternalInput")
    lh = nc.dram_tensor("lh", [NCHUNK, P], F32, kind="ExternalInput")
    out = nc.dram_tensor("out", [NI, MJ], F16, kind="ExternalOutput")

    xt_r = xt.rearrange("(c d) i -> d c i", d=P)
    yt_r = yt.rearrange("(c d) j -> d c j", d=P)

    with tile.TileContext(nc) as tc:
        with (
            tc.tile_pool(name="singles", bufs=1) as singles,
            tc.tile_pool(name="stage", bufs=3) as stage,
            tc.tile_pool(name="sqp", bufs=2) as sqp,
            tc.tile_pool(name="accp", bufs=2, space="PSUM") as accp,
            tc.tile_pool(name="outp", bufs=3) as outp,
            tc.tile_pool(name="tmpp", bufs=3) as tmpp,
            tc.tile_pool(name="mainps", bufs=3, space="PSUM") as mainps,
        ):
            # persistent SBUF tensors
            xs8 = singles.tile([P, NCHUNK, NI], FP8)  # ih * x^T, fp8
            ys8 = singles.tile([P, NCHUNK, MJ], FP8)  # -2 ih * y^T, fp8
            mhx2 = singles.tile([P, ITILES], F32)  # -0.5 * x2, ACT bias
            sx = singles.tile([1, NI], F32)  # -0.5 * x2 row
            ey2rep = singles.tile([P, MJ], F16)  # exp(-0.5*y2) replicated
            ones1 = singles.tile([1, P], BF16)  # replicate-matmul lhsT
            lhs = singles.tile([P, NCHUNK], F32)
            ih = singles.tile([P, NCHUNK], F32)
            ihm2 = singles.tile([P, NCHUNK], F32)
            ihsq = singles.tile([P, NCHUNK], F32)
            mihsq = singles.tile([P, NCHUNK], F32)

            nc.sync.dma_start(out=lhs, in_=lh.rearrange("c d -> d c"))
            nc.scalar.activation(ih, lhs, AF.Exp, scale=-0.5)
            nc.vector.tensor_scalar_mul(ihm2, ih, -2.0)
            # f32r-tagged writes: the BIR verifier requires every operand of
            # an fp32r matmul to be produced as fp32r.
            nc.vector.tensor_mul(ihsq.bitcast(F32R), ih, ih)
            nc.vector.tensor_scalar_mul(mihsq.bitcast(F32R), ihsq, -0.5)
            nc.vector.memset(ones1, 1.0)

            def prep_slab(src_r, dst8, scale, lhsT, s, pfx):
                # One 512-column slab: DMA load all 4 contraction chunks,
                # square (Pool), reduce sum_d lhsT*src^2 into a PSUM chain,
                # scale to fp8. Returns the [1, 512] PSUM row accumulator.
                s0 = s * 512
                sf = stage.tile(
                    [P, NCHUNK, 512], F32, tag=f"{pfx}f", name=f"{pfx}f{s}"
                )
                nc.sync.dma_start(out=sf, in_=src_r[:, :, s0 : s0 + 512])
                sq = sqp.tile(
                    [P, NCHUNK, 512], F32, tag=f"{pfx}sq", name=f"{pfx}sq{s}"
                )
                nc.gpsimd.tensor_mul(sq.bitcast(F32R), sf, sf)
                acc = accp.tile([1, 512], F32, tag="acc", name=f"{pfx}a{s}")
                for c in range(NCHUNK):
                    nc.tensor.matmul(
                        acc,
                        lhsT[:, c : c + 1].bitcast(F32R),
                        sq[:, c, :].bitcast(F32R),
                        start=(c == 0),
                        stop=(c == NCHUNK - 1),
                    )
                for c in range(NCHUNK):
                    nc.vector.tensor_scalar_mul(
                        dst8[:, c, s0 : s0 + 512],
                        sf[:, c, :],
                        scale[:, c : c + 1],
                    )
                return acc

            # ---- y prep: ys8 + replicated exp(-0.5*y2) column factors ----
            ey2row = singles.tile([1, MJ], BF16)
            for s in range(YSLABS):
                s0 = s * 512
                acc = prep_slab(yt_r, ys8, ihm2, ihsq, s, "y")
                nc.scalar.activation(
                    ey2row[0:1, s0 : s0 + 512],
                    acc,
                    AF.Exp,
                    scale=-0.5,
                )
            # replicate ey2row to all 128 partitions with ones-matmuls
            # (PSUM tiles borrowed from the main pool's ring)
            for g in range(MJ // 1024):
                rep = mainps.tile([P, 1024], F32, tag="ps", name=f"rep{g}")
                for h in range(2):
                    r0 = g * 1024 + h * 512
                    nc.tensor.matmul(
                        rep[:, h * 512 : (h + 1) * 512],
                        ones1,
                        ey2row[0:1, r0 : r0 + 512],
                        start=True,
                        stop=True,
                    )
                nc.vector.tensor_copy(
                    ey2rep[:, g * 1024 : (g + 1) * 1024], rep
                )

            # ---- x prep: xs8 + the -0.5*x2 ACT bias column ----
            for s in range(XSLABS):
                s0 = s * 512
                acc = prep_slab(xt_r, xs8, ih, mihsq, s, "x")
                nc.vector.tensor_copy(sx[0:1, s0 : s0 + 512], acc)

            # transpose -0.5*x2 row [1, NI] -> [P, ITILES] for the ACT bias:
            # one column DMA per itile (a single balanced DMA would need >3
            # AP dims, which the DMA engine can't express).
            for it in range(ITILES):
                nc.sync.dma_start(
                    out=mhx2[:, it : it + 1],
                    in_=sx[0:1, it * P : (it + 1) * P],
                )

            # ---- main loop: fp8 DoubleRow matmuls, ACT exp, Pool scale ----
            for it in range(ITILES):
                isl = slice(it * P, (it + 1) * P)
                ot = outp.tile([P, MJ], F16, tag="ot", name=f"ot{it}")
                for jg in range(MJ // 1024):
                    jsl = slice(jg * 1024, (jg + 1) * 1024)
                    ps = mainps.tile(
                        [P, 1024], F32, tag="ps", name=f"ps{it}_{jg}"
                    )
                    for t in range(NPAIR):
                        csl = slice(2 * t, 2 * t + 2)
                        for js in range(2):
                            j0 = jg * 1024 + js * 512
                            nc.tensor.matmul(
                                ps[:, js * 512 : (js + 1) * 512],
                                xs8[:, csl, isl],
                                ys8[:, csl, j0 : j0 + 512],
                                start=(t == 0),
                                stop=(t == NPAIR - 1),
                                perf_mode=DR,
                            )
                    tmp = tmpp.tile([P, 1024], F16, tag="tmp", name=f"t{it}_{jg}")
                    nc.scalar.activation(
                        tmp,
                        ps,
                        AF.Exp,
                        bias=mhx2[:, it : it + 1],
                        scale=-0.5,
                    )
                    nc.gpsimd.tensor_mul(ot[:, jsl], tmp, ey2rep[:, jsl])
                nc.sync.dma_start(out=out[isl, :], in_=ot)

    return _patch_nc(nc)


_NC_CACHE = None

# test.py hooks: set _TRACE to capture a profile; results object stored here.
_TRACE = False
_TRACE_KWARGS = {}
LAST_RESULTS = None


def kernel(x, y, logh):
    global _NC_CACHE, LAST_RESULTS
    x = np.ascontiguousarray(np.asarray(x, dtype=np.float32))
    y = np.ascontiguousarray(np.asarray(y, dtype=np.float32))
    logh = np.ascontiguousarray(np.asarray(logh, dtype=np.float32))
    assert x.shape == (N, D) and y.shape == (M, D) and logh.shape == (D,)

    if _NC_CACHE is None:
        _NC_CACHE = _build_nc()
    nc = _NC_CACHE

    lhm = np.ascontiguousarray(logh.reshape(NCHUNK, P))
    xts = [
        np.ascontiguousarray(x[r * NI : (r + 1) * NI, :].T) for r in range(RG)
    ]
    yts = [
        np.ascontiguousarray(y[q * MJ : (q + 1) * MJ, :].T) for q in range(CG)
    ]
    in_maps = []
    for c in range(N_CORES):
        r, q = divmod(c, CG)
        in_maps.append({"xt": xts[r], "yt": yts[q], "lh": lhm})

    res = run_bass_kernel_spmd(
        nc,
        in_maps,
        core_ids=list(range(N_CORES)),
        trace=_TRACE,
        **_TRACE_KWARGS,
    )
    LAST_RESULTS = res
    full = np.empty((N, M), dtype=np.float32)
    for c in range(N_CORES):
        r, q = divmod(c, CG)
        full[r * NI : (r + 1) * NI, q * MJ : (q + 1) * MJ] = res.results[c][
            "out"
        ].astype(np.float32)
    return full


# revision 13
# speedup vs baseline: 1.7585x; 1.4420x over previous
"""ARD RBF Gram matrix kernel for Trainium2 (8 NeuronCores, SPMD).

K[i, j] = exp(-0.5 * sum_d (x[i,d] - y[j,d])^2 / exp(logh[d]))

Sharding: 2x4 core grid. Core c = (r, q) with r = c // 4, q = c % 4 owns the
output block rows [r*4096, (r+1)*4096) x cols [q*2048, (q+1)*2048). This
minimizes per-core input DMA (8MB of x + 4MB of y vs 2MB + 16MB for pure row
sharding).

Device-side algorithm per core, using the factorization
  K = exp(-0.5*cross - 0.5*x2[i]) * exp(-0.5*y2[j]),  cross = -2 sum ih^2 x y:

  ih      = exp(-0.5 * logh)                       (ACT)
  xs8     = fp8e4(x^T * ih)     [d, c, i] layout   (DVE per-partition scale)
  ys8     = fp8e4(y^T * -2ih)   [d, c, j] layout   (DVE per-partition scale)
  mhx2[i] = -0.5*sum_d ih^2 x^2  (Pool square + f32r reduce matmuls; becomes
                                  the ACT bias after a transpose DMA)
  ey2[j]  = exp(-0.5*sum_d ih^2 y^2)  (ACT exp), replicated to all 128
                                  partitions via ones-matmuls
  psum    = cross  (fp8 DoubleRow matmuls, 256-deep contraction per pass)
  tmp     = exp(-0.5*psum + mhx2[i])   (ACT, PSUM -> SBUF fp16)
  out     = tmp * ey2rep               (DVE fp16 multiply)
  DMA store fp16 to DRAM; host widens fp16 -> fp32 (lossless).

Engine-cost rules learned from traces on this hardware:
  - DVE/Pool ops cost ~1.4us / ~2.4us each nearly independent of size up to
    [128, 4096], so use the fewest, widest ops possible.
  - PE fp8 DoubleRow matmuls stream 2 fp8 columns/cycle; keeping the PE
    stream gap-free matters more than anything (HAM p-state ramp).
  - The aug matmuls of earlier designs cost as much PE streaming time as the
    real DR matmuls, hence the exp-factorization above.

The host side only reshapes/transposes/shards numpy arrays and losslessly
widens the fp16 result; every value-changing floating point operation
happens on device.
"""

import json

import numpy as np

import concourse.bass as bass
import concourse.mybir as mybir
import concourse.tile as tile
from concourse.bass_utils import run_bass_kernel_spmd

N_CORES = 8
N, M, D = 8192, 8192, 512
RG, CG = 2, 4  # core grid: RG row groups x CG col groups
NI = N // RG  # x rows per core (4096)
MJ = M // CG  # y cols per core (2048)
P = 128  # partitions
NCHUNK = D // P  # contraction chunks (4)
NPAIR = NCHUNK // 2  # fp8 DoubleRow chunk pairs (2)
ITILES = NI // P  # i tiles per core (32)
SLABW = 2048  # prep slab width
XSLABS = NI // SLABW  # 2
YSLABS = MJ // SLABW  # 1

F32 = mybir.dt.float32
F32R = mybir.dt.float32r
BF16 = mybir.dt.bfloat16
F16 = mybir.dt.float16
FP8 = mybir.dt.float8e4
AF = mybir.ActivationFunctionType
DR = mybir.MatmulPerfMode.DoubleRow

# ---------------------------------------------------------------------------
# Workaround for this walrus build: only ONE sync-wait condition is allowed
# per instruction ("Too many sync wait commands"). Split excess on_wait
# entries onto preceding NoOps on the same engine (program order preserves
# semantics exactly).
# ---------------------------------------------------------------------------
_WAIT_LIMIT = 1


def _split_excess_waits(bir: dict, limit: int = _WAIT_LIMIT) -> dict:
    # Excess waits are moved onto preceding EventSemaphore instructions,
    # which this walrus accepts with up to TWO wait conditions (ordinary
    # instructions allow only one). Program order preserves semantics.
    counter = 0
    for fn in bir.get("functions", []):
        for bb in fn.get("blocks", []):
            new_insts = []
            for inst in bb.get("instructions", []):
                si = inst.get("sync_info")
                waits = si.get("on_wait") if si else None
                eng = inst.get("engine", "Unassigned")
                if waits and len(waits) > limit and eng != "Unassigned":
                    keep = len(waits) % 2  # odd count: last wait stays put
                    head = waits[: len(waits) - keep]
                    for i in range(0, len(head), 2):
                        counter += 1
                        new_insts.append(
                            {
                                "debug": inst.get("debug", 0),
                                "engine": eng,
                                "ins": [],
                                "outs": [],
                                "name": f"WS-{counter}-{inst['name']}",
                                "opcode": "EventSemaphore",
                                "sync_info": {
                                    "on_update": [],
                                    "on_wait": head[i : i + 2],
                                },
                            }
                        )
                    si["on_wait"] = waits[len(waits) - keep :]
                new_insts.append(inst)
            bb["instructions"] = new_insts
    return bir


def _patch_nc(nc):
    orig = nc.to_json_bytes

    def patched() -> bytes:
        return json.dumps(_split_excess_waits(json.loads(orig()))).encode()

    nc.to_json_bytes = patched
    return nc


# ---------------------------------------------------------------------------
# Device program (identical on all 8 cores; only DRAM contents differ)
# ---------------------------------------------------------------------------


def _build_nc():
    nc = bass.Bass()

    xt = nc.dram_tensor("xt", [D, NI], F32, kind="ExternalInput")
    yt = nc.dram_tensor("yt", [D, MJ], F32, kind="ExternalInput")
    lh = nc.dram_tensor("lh", [NCHUNK, P], F32, kind="ExternalInput")
    out = nc.dram_tensor("out", [NI, MJ], F16, kind="ExternalOutput")

    xt_r = xt.rearrange("(c d) i -> d c i", d=P)
    yt_r = yt.rearrange("(c d) j -> d c j", d=P)

    with tile.TileContext(nc) as tc:
        with (
            tc.tile_pool(name="singles", bufs=1) as singles,
            tc.tile_pool(name="stage", bufs=2) as stage,
            tc.tile_pool(name="sqp", bufs=2) as sqp,
            tc.tile_pool(name="outp", bufs=3) as outp,
            tc.tile_pool(name="tmpp", bufs=3) as tmpp,
        ):
            # persistent SBUF tensors
            xs8 = singles.tile([P, NCHUNK, NI], FP8)  # ih * x^T, fp8
            ys8 = singles.tile([P, NCHUNK, MJ], FP8)  # -2 ih * y^T, fp8
            mhx2 = singles.tile([P, ITILES], F32)  # -0.5 * x2, ACT bias
            sx = singles.tile([1, NI], F32)  # -0.5 * x2 row
            ey2row = singles.tile([1, MJ], BF16)  # exp(-0.5*y2) row
            ey2rep = singles.tile([P, MJ], F16)  # ... replicated
            ones1 = singles.tile([1, P], BF16)  # replicate-matmul lhsT
            lhs = singles.tile([P, NCHUNK], F32)
            ih = singles.tile([P, NCHUNK], F32)
            ihm2 = singles.tile([P, NCHUNK], F32)
            ihsq = singles.tile([P, NCHUNK], F32)
            mihsq = singles.tile([P, NCHUNK], F32)

            nc.sync.dma_start(out=lhs, in_=lh.rearrange("c d -> d c"))
            nc.scalar.activation(ih, lhs, AF.Exp, scale=-0.5)
            nc.vector.tensor_scalar_mul(ihm2, ih, -2.0)
            # f32r-tagged writes: the BIR verifier requires every operand of
            # an fp32r matmul to be produced as fp32r.
            nc.vector.tensor_mul(ihsq.bitcast(F32R), ih, ih)
            nc.vector.tensor_scalar_mul(mihsq.bitcast(F32R), ihsq, -0.5)
            nc.vector.memset(ones1, 1.0)

            # ---- prep: fp8 conversions + row reductions (accp scoped so
            # its PSUM banks are free again before the main loop) ----
            with tc.tile_pool(name="accp", bufs=1, space="PSUM") as accp:

                def prep_slab(src_r, dst8, scale, lhsT, s, pfx):
                    # One 2048-column slab: DMA load all 4 contraction
                    # chunks, square (Pool), reduce sum_d lhsT*src^2 into
                    # four [1, 512] PSUM chains, scale to fp8 (DVE).
                    s0 = s * SLABW
                    sf = stage.tile(
                        [P, NCHUNK, SLABW], F32, tag="sf", name=f"{pfx}f{s}"
                    )
                    nc.sync.dma_start(out=sf, in_=src_r[:, :, s0 : s0 + SLABW])
                    accs = [
                        accp.tile([1, 512], F32, tag=f"a{js}", name=f"{pfx}a{s}_{js}")
                        for js in range(SLABW // 512)
                    ]
                    for c in range(NCHUNK):
                        sq = sqp.tile(
                            [P, SLABW], F32, tag="sq", name=f"{pfx}sq{s}_{c}"
                        )
                        nc.gpsimd.tensor_mul(
                            sq.bitcast(F32R), sf[:, c, :], sf[:, c, :]
                        )
                        for js in range(SLABW // 512):
                            nc.tensor.matmul(
                                accs[js],
                                lhsT[:, c : c + 1].bitcast(F32R),
                                sq[:, js * 512 : (js + 1) * 512].bitcast(F32R),
                                start=(c == 0),
                                stop=(c == NCHUNK - 1),
                            )
                        nc.vector.tensor_scalar_mul(
                            dst8[:, c, s0 : s0 + SLABW],
                            sf[:, c, :],
                            scale[:, c : c + 1],
                        )
                    return accs

                # y: ys8 + exp(-0.5*y2) row (ACT reads the PSUM chains)
                for s in range(YSLABS):
                    accs = prep_slab(yt_r, ys8, ihm2, ihsq, s, "y")
                    for js, acc in enumerate(accs):
                        r0 = s * SLABW + js * 512
                        nc.scalar.activation(
                            ey2row[0:1, r0 : r0 + 512], acc, AF.Exp, scale=-0.5
                        )

                # x: xs8 + -0.5*x2 row (ACT copies PSUM -> SBUF)
                for s in range(XSLABS):
                    accs = prep_slab(xt_r, xs8, ih, mihsq, s, "x")
                    for js, acc in enumerate(accs):
                        r0 = s * SLABW + js * 512
                        nc.scalar.copy(sx[0:1, r0 : r0 + 512], acc)

            # transpose -0.5*x2 row [1, NI] -> [P, ITILES] for the ACT
            # bias: one column DMA per itile (anything wider needs >3 AP
            # dims, which the DMA engine can't express).
            for it in range(ITILES):
                nc.sync.dma_start(
                    out=mhx2[:, it : it + 1],
                    in_=sx[0:1, it * P : (it + 1) * P],
                )

            with tc.tile_pool(name="mainps", bufs=2, space="PSUM") as mainps:
                # replicate ey2row to all 128 partitions with ones-matmuls
                rep = mainps.tile([P, MJ], F32, tag="ps", name="rep")
                for h in range(MJ // 512):
                    nc.tensor.matmul(
                        rep[:, h * 512 : (h + 1) * 512],
                        ones1,
                        ey2row[0:1, h * 512 : (h + 1) * 512],
                        start=True,
                        stop=True,
                    )
                nc.vector.tensor_copy(ey2rep, rep)

                # ---- main loop: fp8 DR matmuls, ACT exp, DVE scale ----
                for it in range(ITILES):
                    isl = slice(it * P, (it + 1) * P)
                    ps = mainps.tile([P, MJ], F32, tag="ps", name=f"ps{it}")
                    for t in range(NPAIR):
                        csl = slice(2 * t, 2 * t + 2)
                        for js in range(MJ // 512):
                            j0 = js * 512
                            nc.tensor.matmul(
                                ps[:, j0 : j0 + 512],
                                xs8[:, csl, isl],
                                ys8[:, csl, j0 : j0 + 512],
                                start=(t == 0),
                                stop=(t == NPAIR - 1),
                                perf_mode=DR,
                            )
                    tmp = tmpp.tile([P, MJ], F16, tag="tmp", name=f"t{it}")
                    nc.scalar.activation(
                        tmp, ps, AF.Exp, bias=mhx2[:, it : it + 1], scale=-0.5
                    )
                    ot = outp.tile([P, MJ], F16, tag="ot", name=f"ot{it}")
                    nc.vector.tensor_mul(ot, tmp, ey2rep)
                    nc.sync.dma_start(out=out[isl, :], in_=ot)

    return _patch_nc(nc)


_NC_CACHE = None

# test.py hooks: set _TRACE to capture a profile; results object stored here.
_TRACE = False
_TRACE_KWARGS = {}
LAST_RESULTS = None


def kernel(x, y, logh):
    global _NC_CACHE, LAST_RESULTS
    x = np.ascontiguousarray(np.asarray(x, dtype=np.float32))
    y = np.ascontiguousarray(np.asarray(y, dtype=np.float32))
    logh = np.ascontiguousarray(np.asarray(logh, dtype=np.float32))
    assert x.shape == (N, D) and y.shape == (M, D) and logh.shape == (D,)

    if _NC_CACHE is None:
        _NC_CACHE = _build_nc()
    nc = _NC_CACHE

    lhm = np.ascontiguousarray(logh.reshape(NCHUNK, P))
    xts = [
        np.ascontiguousarray(x[r * NI : (r + 1) * NI, :].T) for r in range(RG)
    ]
    yts = [
        np.ascontiguousarray(y[q * MJ : (q + 1) * MJ, :].T) for q in range(CG)
    ]
    in_maps = []
    for c in range(N_CORES):
        r, q = divmod(c, CG)
        in_maps.append({"xt": xts[r], "yt": yts[q], "lh": lhm})

    res = run_bass_kernel_spmd(
        nc,
        in_maps,
        core_ids=list(range(N_CORES)),
        trace=_TRACE,
        **_TRACE_KWARGS,
    )
    LAST_RESULTS = res
    full = np.empty((N, M), dtype=np.float32)
    for c in range(N_CORES):
        r, q = divmod(c, CG)
        full[r * NI : (r + 1) * NI, q * MJ : (q + 1) * MJ] = res.results[c][
            "out"
        ].astype(np.float32)
    return full


# revision 15
# speedup vs baseline: 2.1437x; 1.2190x over previous
"""ARD RBF Gram matrix kernel for Trainium2 (8 NeuronCores, SPMD).

K[i, j] = exp(-0.5 * sum_d (x[i,d] - y[j,d])^2 / exp(logh[d]))

Sharding: 2x4 core grid. Core c = (r, q) with r = c // 4, q = c % 4 owns the
output block rows [r*4096, (r+1)*4096) x cols [q*2048, (q+1)*2048). This
minimizes per-core input DMA (8MB of x + 4MB of y vs 2MB + 16MB for pure row
sharding).

Device-side algorithm per core, using the factorization
  K = exp(-0.5*cross - 0.5*x2[i]) * exp(-0.5*y2[j]),  cross = -2 sum ih^2 x y:

  ih      = exp(-0.5 * logh)                       (ACT)
  xs8     = fp8e4(x^T * ih)     [d, c, i] layout   (ACT/DVE per-part. scale)
  ys8     = fp8e4(y^T * -2ih)   [d, c, j] layout
  mhx2[i] = -0.5*sum_d ih^2 x^2  (DVE bf16 square + bf16 reduce matmuls;
                                  becomes the ACT bias via transpose DMAs)
  ey2[j]  = exp(-0.5*sum_d ih^2 y^2)  (ACT exp), replicated to all 128
                                  partitions via ones-matmuls
  psum    = cross  (fp8 DoubleRow matmuls, 256-deep contraction per pass)
  tmp     = exp(-0.5*psum + mhx2[i])   (ACT, PSUM -> SBUF fp16)
  out     = tmp * ey2rep               (DVE fp16 multiply)
  DMA store fp16 to DRAM; host widens fp16 -> fp32 (lossless).

Engine-cost rules learned from traces on this hardware:
  - DVE/Pool elementwise ops have ~1us fixed cost; DVE streams f32/f16 at
    ~2-3 elem/lane/cycle but fp8 writes at ~0.6; Pool is ~2x slower than DVE
    at everything. So: fewest/widest ops, fp8 conversions split ACT (prep) /
    DVE (main-phase slack), squares in bf16 on DVE, tiny row copies on Pool.
  - PE fp8 DoubleRow matmuls stream 2 fp8 columns/cycle; the PE stream must
    stay gap-free or the HAM p-state throttles to ~1.2-1.3 GHz.
  - x-slab 1's fp8 conversion is software-pipelined into the first main-loop
    iterations (its itiles run last), so the main loop starts after only
    y + x-slab-0 prep.

The host side only reshapes/transposes/shards numpy arrays and losslessly
widens the fp16 result; every value-changing floating point operation
happens on device.
"""

import json

import numpy as np

import concourse.bass as bass
import concourse.mybir as mybir
import concourse.tile as tile
from concourse.bass_utils import run_bass_kernel_spmd

N_CORES = 8
N, M, D = 8192, 8192, 512
RG, CG = 2, 4  # core grid: RG row groups x CG col groups
NI = N // RG  # x rows per core (4096)
MJ = M // CG  # y cols per core (2048)
P = 128  # partitions
NCHUNK = D // P  # contraction chunks (4)
NPAIR = NCHUNK // 2  # fp8 DoubleRow chunk pairs (2)
ITILES = NI // P  # i tiles per core (32)
SLABW = 2048  # prep slab width

F32 = mybir.dt.float32
F32R = mybir.dt.float32r
BF16 = mybir.dt.bfloat16
F16 = mybir.dt.float16
FP8 = mybir.dt.float8e4
AF = mybir.ActivationFunctionType
DR = mybir.MatmulPerfMode.DoubleRow

# ---------------------------------------------------------------------------
# Workaround for this walrus build: only ONE sync-wait condition is allowed
# per instruction ("Too many sync wait commands"). Split excess on_wait
# entries onto preceding NoOps on the same engine (program order preserves
# semantics exactly).
# ---------------------------------------------------------------------------
_WAIT_LIMIT = 1


def _split_excess_waits(bir: dict, limit: int = _WAIT_LIMIT) -> dict:
    # Excess waits are moved onto preceding EventSemaphore instructions,
    # which this walrus accepts with up to TWO wait conditions (ordinary
    # instructions allow only one). Program order preserves semantics.
    counter = 0
    for fn in bir.get("functions", []):
        for bb in fn.get("blocks", []):
            new_insts = []
            for inst in bb.get("instructions", []):
                si = inst.get("sync_info")
                waits = si.get("on_wait") if si else None
                eng = inst.get("engine", "Unassigned")
                if waits and len(waits) > limit and eng != "Unassigned":
                    keep = len(waits) % 2  # odd count: last wait stays put
                    head = waits[: len(waits) - keep]
                    for i in range(0, len(head), 2):
                        counter += 1
                        new_insts.append(
                            {
                                "debug": inst.get("debug", 0),
                                "engine": eng,
                                "ins": [],
                                "outs": [],
                                "name": f"WS-{counter}-{inst['name']}",
                                "opcode": "EventSemaphore",
                                "sync_info": {
                                    "on_update": [],
                                    "on_wait": head[i : i + 2],
                                },
                            }
                        )
                    si["on_wait"] = waits[len(waits) - keep :]
                new_insts.append(inst)
            bb["instructions"] = new_insts
    return bir


def _patch_nc(nc):
    orig = nc.to_json_bytes

    def patched() -> bytes:
        return json.dumps(_split_excess_waits(json.loads(orig()))).encode()

    nc.to_json_bytes = patched
    return nc


# ---------------------------------------------------------------------------
# Device program (identical on all 8 cores; only DRAM contents differ)
# ---------------------------------------------------------------------------


def _build_nc():
    nc = bass.Bass()

    xt = nc.dram_tensor("xt", [D, NI], F32, kind="ExternalInput")
    yt = nc.dram_tensor("yt", [D, MJ], F32, kind="ExternalInput")
    lh = nc.dram_tensor("lh", [NCHUNK, P], F32, kind="ExternalInput")
    out = nc.dram_tensor("out", [NI, MJ], F16, kind="ExternalOutput")

    xt_r = xt.rearrange("(c d) i -> d c i", d=P)
    yt_r = yt.rearrange("(c d) j -> d c j", d=P)

    with tile.TileContext(nc) as tc:
        with (
            tc.tile_pool(name="singles", bufs=1) as singles,
            tc.tile_pool(name="stage", bufs=6) as stage,
            tc.tile_pool(name="sqp", bufs=2) as sqp,
            tc.tile_pool(name="outp", bufs=3) as outp,
            tc.tile_pool(name="tmpp", bufs=3) as tmpp,
        ):
            # persistent SBUF tensors
            xs8 = singles.tile([P, NCHUNK, NI], FP8)  # ih * x^T, fp8
            ys8 = singles.tile([P, NCHUNK, MJ], FP8)  # -2 ih * y^T, fp8
            mhx2a = singles.tile([P, ITILES // 2], F32)  # -0.5*x2 bias, slab0
            mhx2b = singles.tile([P, ITILES // 2], F32)  # ... slab 1
            sxa = singles.tile([1, SLABW], F32)  # -0.5 * x2 row, slab 0
            sxb = singles.tile([1, SLABW], F32)  # ... slab 1
            ey2row = singles.tile([1, MJ], BF16)  # exp(-0.5*y2) row
            ey2rep = singles.tile([P, MJ], F16)  # ... replicated
            ones1 = singles.tile([1, P], BF16)  # replicate-matmul lhsT
            lhs = singles.tile([P, NCHUNK], F32)
            ih = singles.tile([P, NCHUNK], F32)
            ihm2 = singles.tile([P, NCHUNK], F32)
            ihsq = singles.tile([P, NCHUNK], BF16)  # ih^2 reduce lhsT
            mihsq = singles.tile([P, NCHUNK], BF16)  # -0.5 ih^2 reduce lhsT

            nc.sync.dma_start(out=lhs, in_=lh.rearrange("c d -> d c"))
            nc.scalar.activation(ih, lhs, AF.Exp, scale=-0.5)
            nc.vector.tensor_scalar_mul(ihm2, ih, -2.0)
            nc.vector.tensor_mul(ihsq, ih, ih)
            nc.vector.tensor_scalar_mul(mihsq, ihsq, -0.5)
            nc.vector.memset(ones1, 1.0)

            held = {}  # x-slab-1 f32 chunk tiles, converted in main phase

            # ---- prep: loads, bf16 squares (DVE), bf16 row-reduce matmuls,
            # fp8 conversions (ACT; x-slab 1 deferred to the main phase) ----
            with tc.tile_pool(name="accp", bufs=1, space="PSUM") as accp:

                def prep_slab(src_r, s0, dst8, scale, lhsT, pfx, defer):
                    accs = [
                        accp.tile([1, 512], F32, tag=f"a{js}", name=f"{pfx}a{js}")
                        for js in range(SLABW // 512)
                    ]
                    for c in range(NCHUNK):
                        sf = stage.tile(
                            [P, SLABW], F32, tag="sf", name=f"{pfx}f{c}"
                        )
                        nc.sync.dma_start(
                            out=sf, in_=src_r[:, c, s0 : s0 + SLABW]
                        )
                        sq = sqp.tile(
                            [P, SLABW], BF16, tag="sq", name=f"{pfx}sq{c}"
                        )
                        nc.vector.tensor_mul(sq, sf, sf)
                        for js in range(SLABW // 512):
                            nc.tensor.matmul(
                                accs[js],
                                lhsT[:, c : c + 1],
                                sq[:, js * 512 : (js + 1) * 512],
                                start=(c == 0),
                                stop=(c == NCHUNK - 1),
                            )
                        if defer:
                            held[c] = sf
                        else:
                            nc.scalar.mul(
                                dst8[:, c, s0 : s0 + SLABW],
                                sf,
                                scale[:, c : c + 1],
                            )
                    return accs

                # y: ys8 + exp(-0.5*y2) row
                accs = prep_slab(yt_r, 0, ys8, ihm2, ihsq, "y", False)
                for js, acc in enumerate(accs):
                    nc.scalar.activation(
                        ey2row[0:1, js * 512 : (js + 1) * 512],
                        acc,
                        AF.Exp,
                        scale=-0.5,
                    )

                # x slabs: xs8 + -0.5*x2 rows (slab 1 conversion deferred)
                for s, sx in ((0, sxa), (1, sxb)):
                    accs = prep_slab(
                        xt_r, s * SLABW, xs8, ih, mihsq, f"x{s}", s == 1
                    )
                    for js, acc in enumerate(accs):
                        nc.scalar.copy(sx[0:1, js * 512 : (js + 1) * 512], acc)

            # transpose -0.5*x2 rows [1, 2048] -> [P, 16] for the ACT bias:
            # one column DMA per itile (anything wider needs >3 AP dims,
            # which the DMA engine can't express).
            for it in range(ITILES // 2):
                nc.sync.dma_start(
                    out=mhx2a[:, it : it + 1],
                    in_=sxa[0:1, it * P : (it + 1) * P],
                )
                nc.sync.dma_start(
                    out=mhx2b[:, it : it + 1],
                    in_=sxb[0:1, it * P : (it + 1) * P],
                )

            with tc.tile_pool(name="mainps", bufs=2, space="PSUM") as mainps:
                # replicate ey2row to all 128 partitions with ones-matmuls
                rep = mainps.tile([P, MJ], F32, tag="ps", name="rep")
                for h in range(MJ // 512):
                    nc.tensor.matmul(
                        rep[:, h * 512 : (h + 1) * 512],
                        ones1,
                        ey2row[0:1, h * 512 : (h + 1) * 512],
                        start=True,
                        stop=True,
                    )
                nc.vector.tensor_copy(ey2rep, rep)

                # ---- main loop: fp8 DR matmuls, ACT exp, DVE scale ----
                for it in range(ITILES):
                    if 2 <= it < 2 + NCHUNK:
                        # deferred x-slab-1 fp8 conversion rides the DVE's
                        # main-phase slack (used from itile 16 onward)
                        c = it - 2
                        nc.vector.tensor_scalar_mul(
                            xs8[:, c, SLABW : 2 * SLABW],
                            held[c],
                            ih[:, c : c + 1],
                        )
                    isl = slice(it * P, (it + 1) * P)
                    mhx2 = mhx2a if it < ITILES // 2 else mhx2b
                    itc = it % (ITILES // 2)
                    ps = mainps.tile([P, MJ], F32, tag="ps", name=f"ps{it}")
                    for t in range(NPAIR):
                        csl = slice(2 * t, 2 * t + 2)
                        for js in range(MJ // 512):
                            j0 = js * 512
                            nc.tensor.matmul(
                                ps[:, j0 : j0 + 512],
                                xs8[:, csl, isl],
                                ys8[:, csl, j0 : j0 + 512],
                                start=(t == 0),
                                stop=(t == NPAIR - 1),
                                perf_mode=DR,
                            )
                    tmp = tmpp.tile([P, MJ], F16, tag="tmp", name=f"t{it}")
                    nc.scalar.activation(
                        tmp,
                        ps,
                        AF.Exp,
                        bias=mhx2[:, itc : itc + 1],
                        scale=-0.5,
                    )
                    ot = outp.tile([P, MJ], F16, tag="ot", name=f"ot{it}")
                    nc.vector.tensor_mul(ot, tmp, ey2rep)
                    nc.sync.dma_start(out=out[isl, :], in_=ot)

    return _patch_nc(nc)


_NC_CACHE = None

# test.py hooks: set _TRACE to capture a profile; results object stored here.
_TRACE = False
_TRACE_KWARGS = {}
LAST_RESULTS = None


def kernel(x, y, logh):
    global _NC_CACHE, LAST_RESULTS
    x = np.ascontiguousarray(np.asarray(x, dtype=np.float32))
    y = np.ascontiguousarray(np.asarray(y, dtype=np.float32))
    logh = np.ascontiguousarray(np.asarray(logh, dtype=np.float32))
    assert x.shape == (N, D) and y.shape == (M, D) and logh.shape == (D,)

    if _NC_CACHE is None:
        _NC_CACHE = _build_nc()
    nc = _NC_CACHE

    lhm = np.ascontiguousarray(logh.reshape(NCHUNK, P))
    xts = [
        np.ascontiguousarray(x[r * NI : (r + 1) * NI, :].T) for r in range(RG)
    ]
    yts = [
        np.ascontiguousarray(y[q * MJ : (q + 1) * MJ, :].T) for q in range(CG)
    ]
    in_maps = []
    for c in range(N_CORES):
        r, q = divmod(c, CG)
        in_maps.append({"xt": xts[r], "yt": yts[q], "lh": lhm})

    res = run_bass_kernel_spmd(
        nc,
        in_maps,
        core_ids=list(range(N_CORES)),
        trace=_TRACE,
        **_TRACE_KWARGS,
    )
    LAST_RESULTS = res
    full = np.empty((N, M), dtype=np.float32)
    for c in range(N_CORES):
        r, q = divmod(c, CG)
        full[r * NI : (r + 1) * NI, q * MJ : (q + 1) * MJ] = res.results[c][
            "out"
        ].astype(np.float32)
    return full
